# revision 12
# baseline (speedup 1.0000x reference)
"""Chaotic LSTM (Lee-oscillator activations) on 8 Trainium2 NeuronCores.

Strategy
--------
Data-parallel over batch: 64 rows -> 8 cores x 8 rows, weights replicated,
no collectives (the time recurrence runs independently per core).

The 25-iteration Lee oscillator is a pure pointwise 1-D function of its
input x.  Numerically (float64 scan over the whole input range):
  lee_tanh(x)    = tanh(x) + r(x)         with |r| <= 4.3e-4  -> drop r
  lee_sigmoid(x) = sigmoid(x) + exp(-50 x^2) * P(x)
where P is smooth on the support |x| <= 0.7 of the Gaussian window; a
degree-10 polynomial fit gives ~2e-3 pointwise and ~1.7e-3 end-to-end
relative error through the full 512-step LSTM (tolerance-safe).

Per-core per-step device work:
  gates^T[2048p, 8b] = Wh_perm^T @ h^T   (64 small matmuls, bf16 weights)
  gates += A_t (precomputed x@Wi+b, bulk matmuls amortized off-path)
  lee via one tanh ACT + Square/Exp ACT + fp32 Horner chain on DVE
  LSTM pointwise update; h fed back as bf16 for the next matmul.

Layout: gate columns permuted to chunk order [i(4) f(4) o(4) g(4)] x 128,
so tiles are [128 partitions = H%128, free = (chunk, batch)]; i/f/o/g of
one hidden index share a partition => all pointwise ops stay lane-local,
and h^T K-slices for the next matmul are direct free-dim slices.
Sigmoid gate columns are pre-scaled by 0.5 in the weights so a single
tanh ACT covers both sigmoid (tanh(g/2) form) and tanh gates.
"""

import os
from contextlib import ExitStack

import numpy as np
import ml_dtypes

import concourse.bass as bass
import concourse.bacc as bacc
import concourse.tile as tile
from concourse import mybir
from concourse.bass_utils import run_bass_kernel_spmd

BF16 = ml_dtypes.bfloat16

BS, S, I, H = 64, 512, 256, 512
NCORES = 8
BPC = BS // NCORES          # 8 batch rows per core
TB = 64                     # timesteps per x@Wi bulk block
NBLK = S // TB

# Horner coefficients of P~(y) = P(2y) (y = g/2), degree 8, fit of
# (lee_sigmoid - sigmoid)/exp(-50 g^2) on |g| <= 0.7 (float64 Chebyshev).
COEF = [0.23918132483959198, 0.11912796646356583, 1.4701021909713745,
        0.00993204116821289, -42.3474235534668, -0.26751959323883057,
        461.5308532714844, 1.1610983610153198, -1703.9658203125]
DEG = len(COEF) - 1


def _build(n_steps: int) -> bass.Bass:
    f32 = mybir.dt.float32
    bf16 = mybir.dt.bfloat16
    AF = mybir.ActivationFunctionType
    OP = mybir.AluOpType

    nblk = (n_steps + TB - 1) // TB

    nc = bacc.Bacc(None, target_bir_lowering=False)
    xt_d = nc.declare_dram_parameter("xt", [128, 2 * 8 * S], f32, isOutput=False)
    whb_d = nc.declare_dram_parameter("whb", [128, 4 * 2048], bf16, isOutput=False)
    wib_d = nc.declare_dram_parameter("wib", [128, 2 * 2048], f32, isOutput=False)
    bv_d = nc.declare_dram_parameter("bvec", [128, 16], f32, isOutput=False)
    h0_d = nc.declare_dram_parameter("h0t", [128, 32], f32, isOutput=False)
    c0_d = nc.declare_dram_parameter("c0t", [128, 32], f32, isOutput=False)
    hout_d = nc.declare_dram_parameter("hout", [S, 128, 32], f32, isOutput=True)
    cout_d = nc.declare_dram_parameter("cout", [128, 32], f32, isOutput=True)

    with tile.TileContext(nc) as tc, ExitStack() as ctx:
        const = ctx.enter_context(tc.tile_pool(name="const", bufs=1))
        apool = ctx.enter_context(tc.tile_pool(name="apool", bufs=2))
        state = ctx.enter_context(tc.tile_pool(name="state", bufs=2))
        work = ctx.enter_context(tc.tile_pool(name="work", bufs=2))
        pg_pool = ctx.enter_context(tc.tile_pool(name="pg", bufs=2, space="PSUM"))
        pa_pool = ctx.enter_context(tc.tile_pool(name="pa", bufs=2, space="PSUM"))

        # resident constants
        xt = const.tile([128, 2 * 8 * S], f32, name="xt_sb")
        whb = const.tile([128, 4 * 2048], bf16, name="whb_sb")
        wib = const.tile([128, 2 * 2048], f32, name="wib_sb")
        bv = const.tile([128, 16], f32, name="bv_sb")
        nc.sync.dma_start(out=xt[:, :], in_=xt_d[:, :])
        nc.sync.dma_start(out=whb[:, :], in_=whb_d[:, :])
        nc.sync.dma_start(out=wib[:, :], in_=wib_d[:, :])
        nc.sync.dma_start(out=bv[:, :], in_=bv_d[:, :])

        # initial state
        c_cur = state.tile([128, 32], f32, tag="c")
        h32_0 = work.tile([128, 32], f32, tag="h32")
        hb_cur = state.tile([128, 32], bf16, tag="hb")
        nc.sync.dma_start(out=c_cur[:, :], in_=c0_d[:, :])
        nc.sync.dma_start(out=h32_0[:, :], in_=h0_d[:, :])
        nc.vector.tensor_copy(hb_cur[:, :], h32_0[:, :])

        a_tiles = {}

        def emit_a_mtile(blk: int, m: int):
            """x@Wi bulk: one [128, 512] m-tile of A for time block blk."""
            if blk not in a_tiles:
                a_tiles[blk] = apool.tile([128, 16 * 512], f32, tag="ablk",
                                          name=f"ablk{blk}")
            pa = pa_pool.tile([128, 512], f32, tag="pa")
            for k2 in range(2):
                nc.tensor.matmul(
                    pa[:, :],
                    wib[:, k2 * 2048 + m * 128:k2 * 2048 + (m + 1) * 128],
                    xt[:, k2 * 4096 + blk * 512:k2 * 4096 + (blk + 1) * 512],
                    start=(k2 == 0), stop=(k2 == 1),
                )
            # psum -> sbuf with per-partition bias add (b already 0.5-scaled);
            # on ScalarE: keeps DVE free for the per-step Horner chain
            nc.scalar.activation(
                a_tiles[blk][:, m * 512:(m + 1) * 512], pa[:, :],
                mybir.ActivationFunctionType.Identity, bias=bv[:, m:m + 1])

        for m in range(16):
            emit_a_mtile(0, m)

        for t in range(n_steps):
            blk, tmin = t // TB, t % TB
            a_cur = a_tiles[blk]

            # ---- gates^T = Wh_perm^T @ h^T : 16 m-tiles x 4 k-tiles ----
            pg = pg_pool.tile([128, 128], f32, tag="pg")
            for m in range(16):
                for k in range(4):
                    nc.tensor.matmul(
                        pg[:, m * 8:(m + 1) * 8],
                        whb[:, k * 2048 + m * 128:k * 2048 + (m + 1) * 128],
                        hb_cur[:, k * 8:(k + 1) * 8],
                        start=(k == 0), stop=(k == 3),
                    )

            # ---- G = psum + A_t ----
            g = work.tile([128, 128], f32, tag="g")
            a_view = a_cur[:, :].rearrange("p (m w) -> p m w", m=16)[
                :, :, tmin * 8:(tmin + 1) * 8]
            nc.vector.scalar_tensor_tensor(
                g[:, :].rearrange("p (m w) -> p m w", m=16),
                pg[:, :].rearrange("p (m w) -> p m w", m=16),
                0.0, a_view, OP.bypass, OP.add)

            gs = g[:, 0:96]          # sigmoid gates i,f,o (pre-scaled: y = g/2)

            # ---- base activations ----
            r = work.tile([128, 128], f32, tag="r")
            nc.scalar.activation(r[:, :], g[:, :], AF.Tanh)
            sq = work.tile([128, 96], f32, tag="sq")
            nc.scalar.activation(sq[:, :], gs, AF.Square)
            dec = work.tile([128, 96], f32, tag="dec")
            nc.scalar.activation(dec[:, :], sq[:, :], AF.Exp, scale=-200.0)
            zb = work.tile([128, 96], f32, tag="zb")
            nc.scalar.activation(zb[:, :], r[:, 0:96], AF.Copy, bias=0.5, scale=0.5)

            # ---- Horner chain for the sigmoid correction ----
            w = work.tile([128, 96], f32, tag="w")
            nc.vector.tensor_scalar(w[:, :], gs, COEF[DEG], COEF[DEG - 1],
                                    OP.mult, OP.add)
            for j in range(DEG - 2, 0, -1):
                nc.vector.scalar_tensor_tensor(w[:, :], w[:, :], COEF[j], gs,
                                               OP.add, OP.mult)
            dv = work.tile([128, 96], f32, tag="dv")
            nc.vector.scalar_tensor_tensor(dv[:, :], w[:, :], COEF[0], dec[:, :],
                                           OP.add, OP.mult)
            z = work.tile([128, 96], f32, tag="z")
            nc.vector.tensor_add(z[:, :], zb[:, :], dv[:, :])

            # ---- LSTM pointwise update ----
            t1 = work.tile([128, 32], f32, tag="t1")
            nc.vector.tensor_mul(t1[:, :], z[:, 0:32], r[:, 96:128])   # i * g
            t2 = work.tile([128, 32], f32, tag="t2")
            nc.vector.tensor_mul(t2[:, :], c_cur[:, :], z[:, 32:64])   # c * f
            c_new = state.tile([128, 32], f32, tag="c")
            nc.vector.tensor_add(c_new[:, :], t1[:, :], t2[:, :])
            tc_t = work.tile([128, 32], f32, tag="tc")
            nc.scalar.activation(tc_t[:, :], c_new[:, :], AF.Tanh)
            h32 = work.tile([128, 32], f32, tag="h32")
            nc.vector.tensor_mul(h32[:, :], z[:, 64:96], tc_t[:, :])   # o * tanh(c)
            hb_new = state.tile([128, 32], bf16, tag="hb")
            nc.vector.tensor_copy(hb_new[:, :], h32[:, :])

            nc.sync.dma_start(out=hout_d[t], in_=h32[:, :])

            c_cur, hb_cur = c_new, hb_new

            # spread next block's A bulk matmuls into this block's PE gaps
            if t % 4 == 0 and blk + 1 < nblk:
                emit_a_mtile(blk + 1, (t % TB) // 4)

        nc.sync.dma_start(out=cout_d[:, :], in_=c_cur[:, :])

    nc.compile()
    return nc


# ---------------------------------------------------------------------------
# host side
# ---------------------------------------------------------------------------

def _perm_and_scale():
    """Permuted gate-column order [i f o g] and 0.5 pre-scale for sigmoid."""
    perm = np.concatenate([
        np.arange(0, 512),            # i
        np.arange(512, 1024),         # f
        np.arange(1536, 2048),        # o
        np.arange(1024, 1536),        # g
    ])
    scale = np.ones(2048, np.float32)
    scale[:1536] = 0.5                # i,f,o sigmoid pre-scale
    return perm, scale


def _prep_shared(Wi, Wh, b):
    perm, scale = _perm_and_scale()
    Wh_p = (Wh[:, perm] * scale).astype(np.float32)
    Wi_p = (Wi[:, perm] * scale).astype(np.float32)
    b_p = (b[perm] * scale).astype(np.float32)
    whb = np.ascontiguousarray(
        Wh_p.reshape(4, 128, 2048).transpose(1, 0, 2).reshape(128, 8192)
    ).astype(BF16)
    wib = np.ascontiguousarray(
        Wi_p.reshape(2, 128, 2048).transpose(1, 0, 2).reshape(128, 4096))
    bv = np.ascontiguousarray(b_p.reshape(16, 128).T)
    return whb, wib, bv


def _tile32(v):
    """[BPC, 512] -> [128, 32] tile: val(h,b) at [h%128, (h//128)*8 + b]."""
    return np.ascontiguousarray(
        v.T.reshape(4, 128, BPC).transpose(1, 0, 2).reshape(128, 4 * BPC)
    ).astype(np.float32)


def _untile32(t):
    """inverse of _tile32: [128, 32] -> [BPC, 512]."""
    return np.ascontiguousarray(
        t.reshape(128, 4, BPC).transpose(2, 1, 0).reshape(BPC, 512))


_NC_CACHE = {}


def kernel(x, Wi, Wh, b, h0, c0):
    out, ht, ct, _ = _run(x, Wi, Wh, b, h0, c0, S)
    return out, ht, ct


def _run(x, Wi, Wh, b, h0, c0, n_steps, **spmd_kwargs):
    if n_steps not in _NC_CACHE:
        _NC_CACHE[n_steps] = _build(n_steps)
    nc = _NC_CACHE[n_steps]

    whb, wib, bv = _prep_shared(np.asarray(Wi, np.float32),
                                np.asarray(Wh, np.float32),
                                np.asarray(b, np.float32))
    x = np.asarray(x, np.float32)
    h0 = np.asarray(h0, np.float32)
    c0 = np.asarray(c0, np.float32)

    in_maps = []
    for ci in range(NCORES):
        xs = x[ci * BPC:(ci + 1) * BPC]                # [8, 512, 256]
        xt = np.ascontiguousarray(
            xs.transpose(2, 1, 0).reshape(256, S * BPC)
            .reshape(2, 128, S * BPC).transpose(1, 0, 2).reshape(128, 2 * S * BPC))
        in_maps.append({
            "xt": xt, "whb": whb, "wib": wib, "bvec": bv,
            "h0t": _tile32(h0[ci * BPC:(ci + 1) * BPC]),
            "c0t": _tile32(c0[ci * BPC:(ci + 1) * BPC]),
        })

    res = run_bass_kernel_spmd(nc, in_maps, core_ids=list(range(NCORES)),
                               **spmd_kwargs)

    out = np.empty((BS, S, H), np.float32)
    ct = np.empty((BS, H), np.float32)
    for ci in range(NCORES):
        hout = np.asarray(res.results[ci]["hout"], np.float32)   # [S, 128, 32]
        out[ci * BPC:(ci + 1) * BPC] = np.ascontiguousarray(
            hout.reshape(S, 128, 4, BPC).transpose(3, 0, 2, 1).reshape(BPC, S, H))
        ct[ci * BPC:(ci + 1) * BPC] = _untile32(
            np.asarray(res.results[ci]["cout"], np.float32))
    ht = out[:, n_steps - 1, :].copy()
    return out, ht, ct, res
